# revision 20
# baseline (speedup 1.0000x reference)
"""Trainium2 Bass kernel for nn_Decoder_48378511622552 (GNN message passing decoder).

Strategy (8 NeuronCores, graph-partitioned):
  - Coarse nodes are bin-packed into 128-slot "windows" (49 windows/core) so that
    every window owns <= 128*T_W edges; edges are sharded by destination node and
    laid out window-padded, giving a fully static segment-sum schedule
    (one-hot matmuls accumulating in PSUM per window).
  - Node/edge features are carried between layers as LayerNorm-normalized "z"
    (gamma/beta folded into downstream weights on the host).
  - x[src]/x[dst] gathers: indirect DMA from a replicated bf16 node table
    (rebuilt each layer via AllGather), then PE transposes to feat-major.
  - ELU is computed as  max(p, exp(min(p,1)-1)) - 1  with the +-1 shifts folded
    into biases on the host.
  - MLP1 runs feat-major (weights stationary); MLP2 runs "flipped" (activation
    tiles stationary) to produce natural-layout output for LayerNorm; the
    LN residual is an extra diag(gamma) matmul block.

kernel(**inputs) takes FULL inputs, shards on host, runs one Bass program
SPMD on 8 cores, and reassembles the full output.
"""

import math
import os
import sys
from dataclasses import dataclass, field

import numpy as np

sys.path.insert(0, "/opt/trn_rl_repo")

import concourse.bacc as bacc
import concourse.bass as bass
import concourse.mybir as mybir
import concourse.tile as tile
from concourse.bass import IndirectOffsetOnAxis
from concourse.bass_utils import run_bass_kernel_spmd
from concourse.masks import make_identity

F32 = mybir.dt.float32
BF16 = mybir.dt.bfloat16
I32 = mybir.dt.int32
AX = mybir.AluOpType
AF = mybir.ActivationFunctionType

H = 128
OUT_DIM = 3
P = 128


@dataclass
class Cfg:
    n_cores: int = 8
    W: int = 49          # coarse node windows per core (128 slots each)
    T_w: int = 9         # edge tiles (128 edges) per window
    n_inact_slot: int = 12672  # inactive fine slots per core (multiple of 128)

    @property
    def n_slot(self):
        return self.W * 128

    @property
    def e_slot(self):
        return self.W * self.T_w * 128

    @property
    def f_slot(self):
        s = self.n_slot + self.n_inact_slot
        assert s % 512 == 0
        return s

    @property
    def e_chunks(self):
        return self.W * self.T_w

    @property
    def e_slot_pad(self):
        return ((self.e_chunks + 3) // 4) * 512

    @property
    def n_slot_pad(self):
        return ((self.n_slot + 511) // 512) * 512

    @property
    def tbl_rows(self):
        return self.n_cores * self.n_slot


FULL = Cfg()


# ----------------------------------------------------------------------------
# Host-side: weight folding
# ----------------------------------------------------------------------------

def _fold_weights(inp, cfg: Cfg):
    """Fold LN gammas/betas and ELU shifts into the MLP weights.

    Layer l in 0..3 (coarse0, coarse1, fine0, fine1), weight set w = l % 2.
    Edge-side input fold (g_e, b_e): LN params of the z the edge MLP consumes.
    Node-side input fold (g_x, b_x): LN params of the x-carrier consumed.
    """
    f = {}
    eW0 = np.asarray(inp["eW0"], np.float64)
    eb0 = np.asarray(inp["eb0"], np.float64)
    eW1 = np.asarray(inp["eW1"], np.float64)
    eb1 = np.asarray(inp["eb1"], np.float64)
    eg = np.asarray(inp["eg"], np.float64)
    ebt = np.asarray(inp["ebt"], np.float64)
    nW0 = np.asarray(inp["nW0"], np.float64)
    nb0 = np.asarray(inp["nb0"], np.float64)
    nW1 = np.asarray(inp["nW1"], np.float64)
    nb1 = np.asarray(inp["nb1"], np.float64)
    ng = np.asarray(inp["ng"], np.float64)
    nbt = np.asarray(inp["nbt"], np.float64)
    ones = np.ones(H)
    zeros = np.zeros(H)

    # per-layer input folds
    g_e = [ones, eg[0], eg[1], eg[0]]
    b_e = [zeros, ebt[0], ebt[1], ebt[0]]
    ug = np.asarray(inp["ug"], np.float64)
    ubt = np.asarray(inp["ubt"], np.float64)
    g_x = [ones, ng[0], ug, ng[0]]
    b_x = [zeros, nbt[0], ubt, nbt[0]]

    for l in range(4):
        w = l % 2
        f[f"EW0S{l}"] = (g_x[l][:, None] * eW0[w][0:H]).astype(np.float32)
        f[f"EW0D{l}"] = (g_x[l][:, None] * eW0[w][H : 2 * H]).astype(np.float32)
        f[f"EW0Z{l}"] = (g_e[l][:, None] * eW0[w][2 * H : 3 * H]).astype(np.float32)
        f[f"EB0{l}"] = (
            eb0[w]
            + b_x[l] @ eW0[w][0:H]
            + b_x[l] @ eW0[w][H : 2 * H]
            + b_e[l] @ eW0[w][2 * H : 3 * H]
            + 1.0
        ).astype(np.float32)
        f[f"EW1{l}"] = eW1[w].astype(np.float32)
        # t2 = (t1' - 1) @ eW1 + eb1 ; y = z_prev*g_e + b_e + t2
        f[f"EB1{l}"] = (eb1[w] - eW1[w].sum(axis=0) + b_e[l]).astype(np.float32)
        f[f"GDE{l}"] = np.diag(g_e[l]).astype(np.float32)

        # node side; agg fold uses THIS layer's edge LN params (eg[w], ebt[w])
        f[f"NW0X{l}"] = (g_x[l][:, None] * nW0[w][0:H]).astype(np.float32)
        f[f"NW0A{l}"] = (eg[w][:, None] * nW0[w][H : 2 * H]).astype(np.float32)
        f[f"NMASKV{l}"] = (ebt[w] @ nW0[w][H : 2 * H]).astype(np.float32)
        f[f"NB0{l}"] = (nb0[w] + b_x[l] @ nW0[w][0:H] + 1.0).astype(np.float32)
        f[f"NW1{l}"] = nW1[w].astype(np.float32)
        f[f"NB1{l}"] = (nb1[w] - nW1[w].sum(axis=0) + b_x[l]).astype(np.float32)
        f[f"GDX{l}"] = np.diag(g_x[l]).astype(np.float32)

    dW0 = np.asarray(inp["dW0"], np.float64)
    db0 = np.asarray(inp["db0"], np.float64)
    dW1 = np.asarray(inp["dW1"], np.float64)
    db1 = np.asarray(inp["db1"], np.float64)
    uW0 = np.asarray(inp["uW0"], np.float64)
    ub0 = np.asarray(inp["ub0"], np.float64)
    uW1 = np.asarray(inp["uW1"], np.float64)
    ub1 = np.asarray(inp["ub1"], np.float64)
    oW = np.asarray(inp["oW"], np.float64)
    ob = np.asarray(inp["ob"], np.float64)

    f["DW0"] = dW0.astype(np.float32)  # [2, H]
    f["DB0"] = (db0 + 1.0).astype(np.float32)
    f["DW1"] = dW1.astype(np.float32)
    f["DB1"] = (db1 - dW1.sum(axis=0) + 1.0).astype(np.float32)
    f["UW0E"] = uW0[0:H].astype(np.float32)  # consumes eac' = eac + 1
    f["UW0X"] = (ng[1][:, None] * uW0[H : 2 * H]).astype(np.float32)
    f["UB0"] = (
        ub0 + nbt[1] @ uW0[H : 2 * H] - uW0[0:H].sum(axis=0) + 1.0
    ).astype(np.float32)
    f["UW1"] = uW1.astype(np.float32)
    f["UB1"] = (ub1 - uW1.sum(axis=0) + 1.0).astype(np.float32)
    # output head consumes fine z with (ng[1], nbt[1])
    f["OWP"] = (ng[1][:, None] * oW).astype(np.float32)
    f["OBP"] = (nbt[1] @ oW + ob + 1.0).astype(np.float32)
    return f


# blobs: pack [128,128] weights along free dim; rows into a [1, X] blob
W128_BF16 = [f"{n}{l}" for l in range(4) for n in ("EW0S", "EW0D", "EW0Z", "GDE", "NW0A")] + ["UW0X"]
W128_F32 = [f"{n}{l}" for l in range(4) for n in ("EW1", "NW0X", "NW1", "GDX")] + [
    "DW1", "UW0E", "UW1",
]
ROWS_F32 = [f"{n}{l}" for l in range(4) for n in ("EB0", "EB1", "NMASKV", "NB0", "NB1")] + [
    "DB0", "DB1", "UB0", "UB1",
]


# ----------------------------------------------------------------------------
# Host-side: graph partitioning / sharding
# ----------------------------------------------------------------------------

def _partition(inp, cfg: Cfg, n_c, n_f, e_c):
    """Bin-pack coarse nodes into (core, window, slot); shard + pad edges."""
    rng = np.random  # deterministic given input
    ei = np.asarray(inp["edge_index_c"])
    src = np.asarray(ei[0], np.int64)
    dst = np.asarray(ei[1], np.int64)
    deg = np.bincount(dst, minlength=n_c)

    n_windows = cfg.n_cores * cfg.W
    cap_e = cfg.T_w * 128
    # greedy balanced packing: big-degree nodes first, into the least-loaded
    # window that still has a free node slot
    order = np.argsort(-deg, kind="stable")
    import heapq

    heap = [(0, w) for w in range(n_windows)]
    heapq.heapify(heap)
    win_nodes = [[] for _ in range(n_windows)]
    win_load = np.zeros(n_windows, np.int64)
    stash = []
    for nid in order:
        d = int(deg[nid])
        while True:
            load, w = heapq.heappop(heap)
            if len(win_nodes[w]) < 128:
                break
            stash.append((load, w))
        for it in stash:
            heapq.heappush(heap, it)
        stash.clear()
        win_nodes[w].append(int(nid))
        win_load[w] += d
        heapq.heappush(heap, (int(win_load[w]), w))
    if win_load.max() > cap_e:
        raise RuntimeError(
            f"window overflow: {win_load.max()} > {cap_e}; increase T_w"
        )

    # node -> (core, slot index within core), global table row
    node_core = np.empty(n_c, np.int32)
    node_slot = np.empty(n_c, np.int32)  # slot within core [0, n_slot)
    for w in range(n_windows):
        c, wl = divmod(w, cfg.W)
        for s, nid in enumerate(win_nodes[w]):
            node_core[nid] = c
            node_slot[nid] = wl * 128 + s
    tbl_row = node_core.astype(np.int64) * cfg.n_slot + node_slot  # [n_c]

    # edge placement: edge -> (window of dst), position within window
    e_core = node_core[dst]
    e_win = tbl_row[dst] // 128  # global window id = core*W + wl
    # stable ordering of edges per window
    order_e = np.argsort(e_win, kind="stable")
    pos_in_win = np.zeros(e_c, np.int64)
    counts_win = np.zeros(n_windows, np.int64)
    ew_sorted = e_win[order_e]
    # compute position of each edge within its window
    start = 0
    while start < e_c:
        w = ew_sorted[start]
        end = start
        while end < e_c and ew_sorted[end] == w:
            end += 1
        pos_in_win[order_e[start:end]] = np.arange(end - start)
        counts_win[w] = end - start
        start = end

    # stream position of each edge: core-local window wl, slot = pos
    wl_of_edge = e_win % cfg.W
    stream_pos = wl_of_edge * cap_e + pos_in_win  # within core [0, e_slot)

    return dict(
        node_core=node_core,
        node_slot=node_slot,
        tbl_row=tbl_row,
        e_core=e_core,
        stream_pos=stream_pos,
        deg=deg,
        src=src,
        dst=dst,
    )


def _make_core_inputs(inp, cfg: Cfg, part, folds):
    import ml_dtypes

    bf16 = ml_dtypes.bfloat16
    n_c = part["node_core"].shape[0]
    x = np.asarray(inp["x"], np.float32)
    ea = np.asarray(inp["edge_attr"], np.float32)
    pos_c = np.asarray(inp["pos_c"], np.float32)
    pos_f = np.asarray(inp["pos_f"], np.float32)
    clusters = np.asarray(inp["clusters"], np.int64)
    n_f = clusters.shape[0]

    bf = np.concatenate([folds[n] for n in W128_BF16], axis=1).astype(bf16)
    f32b = np.concatenate([folds[n] for n in W128_F32], axis=1).astype(np.float32)
    rows = np.concatenate(
        [folds[n][None, :] for n in ROWS_F32], axis=1
    ).astype(np.float32)
    dw0 = folds["DW0"].astype(np.float32)  # [2,128]
    owp = folds["OWP"].astype(np.float32)  # [128,3]
    obp = folds["OBP"][None, :].astype(np.float32)  # [1,3]

    # global table of initial x (bf16), in slot order
    tbl0 = np.zeros((cfg.tbl_rows, H), bf16)
    tbl0[part["tbl_row"]] = x[np.arange(n_c)].astype(bf16)

    # fine nodes: active ones = coarse ids (same slot), inactive split evenly
    f_act_mask = np.arange(n_f) < n_c  # fine node f active iff f < n_c
    fine_core = np.empty(n_f, np.int32)
    fine_slot = np.empty(n_f, np.int64)
    fine_core[:n_c] = part["node_core"]
    fine_slot[:n_c] = part["node_slot"]
    inact_ids = np.arange(n_c, n_f)
    n_inact = inact_ids.shape[0]
    per = cfg.n_inact_slot
    assert n_inact <= cfg.n_cores * per
    # round-robin chunks
    ic = np.minimum(np.arange(n_inact) // max(1, math.ceil(n_inact / cfg.n_cores)), cfg.n_cores - 1)
    fine_core[inact_ids] = ic
    off = np.zeros(cfg.n_cores, np.int64)
    slots = np.empty(n_inact, np.int64)
    for i, c in enumerate(ic):
        slots[i] = cfg.n_slot + off[c]
        off[c] += 1
    assert off.max() <= per
    fine_slot[inact_ids] = slots

    core_inputs = []
    out_map = np.full((cfg.n_cores, cfg.f_slot), -1, np.int64)
    for f in range(n_f):
        out_map[fine_core[f], fine_slot[f]] = f

    for c in range(cfg.n_cores):
        m = part["e_core"] == c
        sp = part["stream_pos"][m]
        esrc = part["src"][m]
        edst = part["dst"][m]
        eidx = np.where(m)[0]

        idx_src = np.zeros(cfg.e_slot_pad, np.int32)
        idx_dst = np.zeros(cfg.e_slot_pad, np.int32)
        dstloc = np.full(cfg.e_slot_pad, -1, np.int32)
        zt0 = np.zeros((cfg.e_slot_pad, H), np.float32)
        idx_src[sp] = part["tbl_row"][esrc]
        idx_dst[sp] = part["tbl_row"][edst]
        dstloc[sp] = part["node_slot"][edst] % 128
        zt0[sp] = ea[eidx]

        # per-slot counts for own nodes
        own = part["node_core"] == c
        cnt = np.zeros(cfg.n_slot, np.float32)
        cnt_idx = part["node_slot"][own]
        cnt_val = part["deg"][own]
        cnt[cnt_idx] = cnt_val
        recip = 1.0 / np.maximum(cnt, 1.0)
        bmask = (cnt > 0).astype(np.float32)

        zx0 = np.zeros((cfg.n_slot, H), np.float32)
        zx0[part["node_slot"][own]] = x[own]

        # fine-side
        mf = fine_core == c
        fs = fine_slot[mf]
        fid = np.where(mf)[0]
        idx_cl = np.zeros(cfg.f_slot, np.int32)
        pct = np.zeros((cfg.f_slot, 2), np.float32)
        pft = np.zeros((cfg.f_slot, 2), np.float32)
        idx_cl[fs] = part["tbl_row"][clusters[fid]]
        pct[fs] = pos_c[clusters[fid]]
        pft[fs] = pos_f[fid]

        ci = {
            "IDX_SRC": idx_src.reshape(-1, 128).T.copy(),
            "IDX_DST": idx_dst.reshape(-1, 128).T.copy(),
            "DSTLOC": dstloc.reshape(-1, 128).T.astype(np.float32).copy(),
            "RECIP": recip.reshape(cfg.W, 128).T.copy(),
            "BMASK": np.pad(bmask, (0, cfg.n_slot_pad - cfg.n_slot))[None, :].copy(),
            "ZT0": zt0.T.astype(bf16).copy(),
            "ZX0T": zx0.T.copy(),
            "TBL0": tbl0,
            "IDX_CL": idx_cl.reshape(-1, 128).T.copy(),
            "PCT": pct.T.copy(),
            "PFT": pft.T.copy(),
            "WBF16": bf,
            "WF32": f32b,
            "WROWS": rows,
            "DW0": dw0,
            "OWP": owp,
            "OBP": obp,
        }
        core_inputs.append(ci)

    return core_inputs, out_map


# ----------------------------------------------------------------------------
# Device program
# ----------------------------------------------------------------------------

def _woff_bf16(name):
    return W128_BF16.index(name) * 128


def _woff_f32(name):
    return W128_F32.index(name) * 128


def _roff(name):
    return ROWS_F32.index(name) * 128


class Prog:
    def __init__(self, cfg: Cfg):
        self.cfg = cfg
        nc = bacc.Bacc(
            "TRN2",
            target_bir_lowering=False,
            debug=False,
            num_devices=cfg.n_cores,
        )
        self.nc = nc
        c = cfg
        ET = c.e_slot // 512  # 512-edge tiles per layer
        self.in_specs = {}

        def inp(name, shape, dt):
            t = nc.dram_tensor(name, list(shape), dt, kind="ExternalInput")
            self.in_specs[name] = (shape, dt)
            return t

        IDX_SRC = inp("IDX_SRC", (128, c.e_slot_pad // 128), I32)
        IDX_DST = inp("IDX_DST", (128, c.e_slot_pad // 128), I32)
        DSTLOC = inp("DSTLOC", (128, c.e_slot_pad // 128), F32)
        RECIP = inp("RECIP", (128, c.W), F32)
        BMASK = inp("BMASK", (1, c.n_slot_pad), F32)
        ZT0 = inp("ZT0", (128, c.e_slot_pad), BF16)
        ZX0T = inp("ZX0T", (128, c.n_slot), F32)
        TBL0 = inp("TBL0", (c.tbl_rows, H), BF16)
        IDX_CL = inp("IDX_CL", (128, c.f_slot // 128), I32)
        PCT = inp("PCT", (2, c.f_slot), F32)
        PFT = inp("PFT", (2, c.f_slot), F32)
        WBF16 = inp("WBF16", (128, len(W128_BF16) * 128), BF16)
        WF32 = inp("WF32", (128, len(W128_F32) * 128), F32)
        WROWS = inp("WROWS", (1, len(ROWS_F32) * 128), F32)
        DW0 = inp("DW0", (2, 128), F32)
        OWP = inp("OWP", (128, OUT_DIM), F32)
        OBP = inp("OBP", (1, OUT_DIM), F32)

        OUT = nc.dram_tensor("OUT", [c.f_slot, OUT_DIM], F32, kind="ExternalOutput")

        # internal DRAM
        TBL = nc.dram_tensor("TBL", [c.tbl_rows, H], BF16)
        AGSRC = nc.dram_tensor("AGSRC", [c.n_slot, H], BF16)
        ZXT_A = nc.dram_tensor("ZXT_A", [128, c.n_slot], F32)
        ZXT_B = nc.dram_tensor("ZXT_B", [128, c.n_slot], F32)
        ZXFT_A = nc.dram_tensor("ZXFT_A", [128, c.f_slot], F32)
        ZXFT_B = nc.dram_tensor("ZXFT_B", [128, c.f_slot], F32)
        ZT_D = nc.dram_tensor("ZT_D", [128, c.e_slot_pad], BF16)

        # persistent SBUF
        AGGT = nc.alloc_sbuf_tensor("AGGT_res", [128, c.n_slot_pad], BF16)
        WB = nc.alloc_sbuf_tensor("WB_sb", [128, len(W128_BF16) * 128], BF16)
        WF = nc.alloc_sbuf_tensor("WF_sb", [128, len(W128_F32) * 128], F32)
        WR = nc.alloc_sbuf_tensor("WR_sb", [1, len(ROWS_F32) * 128], F32)
        DW0_sb = nc.alloc_sbuf_tensor("DW0_sb", [2, 128], F32)
        OWP_sb = nc.alloc_sbuf_tensor("OWP_sb", [128, OUT_DIM], F32)
        OBP_sb = nc.alloc_sbuf_tensor("OBP_sb", [1, OUT_DIM], F32)
        IOTA = nc.alloc_sbuf_tensor("IOTA_sb", [128, 128], I32)
        IOTA_F = nc.alloc_sbuf_tensor("IOTAF_sb", [128, 128], F32)
        ID_F = nc.alloc_sbuf_tensor("ID_F", [128, 128], F32)
        ID_B = nc.alloc_sbuf_tensor("ID_B", [128, 128], BF16)
        ONES_R = nc.alloc_sbuf_tensor("ONES_R", [1, 512], F32)
        RECIP_sb = nc.alloc_sbuf_tensor("RECIP_sb", [128, c.W], F32)
        NEG1 = nc.alloc_sbuf_tensor("NEG1", [128, 1], F32)
        EPSB = nc.alloc_sbuf_tensor("EPSB", [128, 1], F32)
        BMASK_sb = nc.alloc_sbuf_tensor("BMASK_sb", [1, c.n_slot_pad], F32)

        replica = [list(range(c.n_cores))]

        with tile.TileContext(nc) as tc:
            self._build(tc, locals())
        nc.compile()

    # ------------------------------------------------------------------
    def _build(self, tc, T):
        nc = self.nc
        c = self.cfg
        cE = c.e_slot

        # pools
        import contextlib

        ctx = contextlib.ExitStack()
        sb = ctx.enter_context(tc.tile_pool(name="sb", bufs=3))
        sb2 = ctx.enter_context(tc.tile_pool(name="sb2", bufs=2))
        ps = ctx.enter_context(tc.tile_pool(name="ps", bufs=2, space="PSUM"))
        psT = ctx.enter_context(tc.tile_pool(name="psT", bufs=2, space="PSUM"))
        psY = ctx.enter_context(tc.tile_pool(name="psY", bufs=2, space="PSUM"))
        psA = ctx.enter_context(tc.tile_pool(name="psA", bufs=2, space="PSUM"))
        stat = ctx.enter_context(tc.tile_pool(name="stat", bufs=4))
        zwin = ctx.enter_context(tc.tile_pool(name="zwin", bufs=3))

        # --- init: constants, weights, resident ZT ---
        nc.gpsimd.iota(T["IOTA"][:], pattern=[[1, 128]], base=0, channel_multiplier=0)
        nc.vector.tensor_copy(T["IOTA_F"][:], T["IOTA"][:])
        make_identity(nc, T["ID_F"][:])
        nc.vector.tensor_copy(T["ID_B"][:], T["ID_F"][:])
        nc.vector.memset(T["ONES_R"][:], 1.0)
        nc.vector.memset(T["NEG1"][:], -1.0)
        nc.vector.memset(T["EPSB"][:], 1e-5)
        nc.sync.dma_start(T["WB"][:], T["WBF16"][:])
        nc.sync.dma_start(T["WF"][:], T["WF32"][:])
        nc.sync.dma_start(T["WR"][:], T["WROWS"][:])
        nc.sync.dma_start(T["DW0_sb"][:], T["DW0"][:])
        nc.sync.dma_start(T["OWP_sb"][:], T["OWP"][:])
        nc.sync.dma_start(T["OBP_sb"][:], T["OBP"][:])
        nc.sync.dma_start(T["RECIP_sb"][:], T["RECIP"][:])
        nc.sync.dma_start(T["BMASK_sb"][:], T["BMASK"][:])

        def wb(name):
            o = _woff_bf16(name)
            return T["WB"][:, o : o + 128]

        def wf(name):
            o = _woff_f32(name)
            return T["WF"][:, o : o + 128]

        def wr(name):
            o = _roff(name)
            return T["WR"][:, o : o + 128]

        self._wb, self._wf, self._wr = wb, wf, wr

        # === the 4 layers ===
        # layer tables: gather source for x
        for l in range(4):
            xtbl = T["TBL0"] if l == 0 else T["TBL"]
            self._edge_phase(tc, T, l, xtbl, sb, sb2, ps, psT, psY, psA, stat, zwin)
            if l < 2:
                self._node_phase_coarse(tc, T, l, sb, sb2, ps, psT, psY, stat)
                self._allgather(tc, T)
            elif l == 2:
                self._node_phase_fine(tc, T, l, sb, sb2, ps, psT, psY, stat, last=False)
                self._allgather(tc, T)
            else:
                self._node_phase_fine(tc, T, l, sb, sb2, ps, psT, psY, stat, last=True)
            if l == 1:
                self._upsample(tc, T, sb, sb2, ps, psT, psY, stat)
                self._allgather(tc, T)

        ctx.close()

    # ------------------------------------------------------------------
    def _ln_block(self, T, stat, sb, y_ps, nat_w, z_out_ap, sums, mus, rstds, j):
        """LayerNorm on a natural [nat_w,128] psum tile -> z (written to z_out_ap).

        sums/mus/rstds are [128, >=j+1] stat tiles; uses column j.
        Returns y_sb (natural f32 SBUF tile)."""
        nc = self.nc
        y_sb = sb.tile([128, 128], F32, tag="y_sb")
        nc.vector.tensor_scalar(
            y_sb[:nat_w, :], y_ps[:nat_w, :], 1.0, 0.0, AX.mult, AX.add,
            accum_out=sums[:nat_w, j : j + 1],
        )
        nc.scalar.mul(mus[:nat_w, j : j + 1], sums[:nat_w, j : j + 1], 1.0 / 128.0)
        sq = sb.tile([128, 128], F32, tag="sq_scratch")
        ssq = stat.tile([128, 1], F32, tag="ssq")
        nc.scalar.activation(
            sq[:nat_w, :], y_sb[:nat_w, :], AF.Square,
            bias=mus[:nat_w, j : j + 1], scale=-1.0,
        )
        nc.vector.tensor_reduce(ssq[:nat_w, :], sq[:nat_w, :], axis=mybir.AxisListType.X, op=AX.add)
        std = stat.tile([128, 1], F32, tag="std")
        nc.scalar.activation(
            std[:nat_w, :], ssq[:nat_w, :], AF.Sqrt, bias=T["EPSB"][:, :], scale=1.0 / 128.0
        )
        nc.vector.reciprocal(rstds[:nat_w, j : j + 1], std[:nat_w, :])
        nc.vector.tensor_scalar(
            z_out_ap, y_sb[:nat_w, :],
            mus[:nat_w, j : j + 1], rstds[:nat_w, j : j + 1],
            AX.subtract, AX.mult,
        )
        return y_sb

    def _elu(self, T, sb, p_ps, w, tag):
        """ELU' = max(p, exp(min(p,1)-1)) on [128, w] psum -> f32 SBUF tile."""
        nc = self.nc
        m = sb.tile([128, 512], F32, tag="elu_m")
        nc.vector.tensor_scalar(m[:, :w], p_ps[:, :w], 1.0, None, AX.min)
        e = sb.tile([128, 512], F32, tag="elu_e")
        nc.scalar.activation(e[:, :w], m[:, :w], AF.Exp, bias=T["NEG1"][:, :], scale=1.0)
        t1 = sb.tile([128, 512], F32, tag="elu_t")
        nc.vector.tensor_tensor(t1[:, :w], e[:, :w], p_ps[:, :w], AX.max)
        return t1

    # ------------------------------------------------------------------
    def _edge_phase(self, tc, T, l, xtbl, sb, sb2, ps, psT, psY, psA, stat, zwin):
        nc = self.nc
        c = self.cfg
        wb, wf, wr = self._wb, self._wf, self._wr
        ET = c.e_slot_pad // 512
        n_chunks = c.e_slot_pad // 128
        zwin_tiles = {}

        # staged idx columns: load 64 chunks (=16 tiles) worth at a time
        STG = 64

        agg_ps = {}
        for t in range(ET):
            c0 = 4 * t  # first 128-chunk of this tile
            if c0 % STG == 0:
                stg_w = min(STG, n_chunks - c0)
                isrc_sb = sb2.tile([128, STG], I32, tag="isrc")
                idst_sb = sb2.tile([128, STG], I32, tag="idst")
                dloc_sb = sb2.tile([128, STG], F32, tag="dloc")
                nc.sync.dma_start(isrc_sb[:, :stg_w], T["IDX_SRC"][:, c0 : c0 + stg_w])
                nc.sync.dma_start(idst_sb[:, :stg_w], T["IDX_DST"][:, c0 : c0 + stg_w])
                nc.sync.dma_start(dloc_sb[:, :stg_w], T["DSTLOC"][:, c0 : c0 + stg_w])
            so = c0 % STG

            # gathers (natural bf16) then PE-transpose to feat-major
            gs = sb.tile([128, 4 * 128], BF16, tag="g_src")
            gd = sb.tile([128, 4 * 128], BF16, tag="g_dst")
            for j in range(4):
                if os.environ.get("KNOGATHER"):
                    nc.sync.dma_start(gs[:, 128 * j : 128 * (j + 1)], xtbl[0:128, :])
                    nc.sync.dma_start(gd[:, 128 * j : 128 * (j + 1)], xtbl[0:128, :])
                    continue
                nc.gpsimd.indirect_dma_start(
                    out=gs[:, 128 * j : 128 * (j + 1)],
                    out_offset=None,
                    in_=xtbl[:],
                    in_offset=IndirectOffsetOnAxis(ap=isrc_sb[:, so + j : so + j + 1], axis=0),
                )
                nc.gpsimd.indirect_dma_start(
                    out=gd[:, 128 * j : 128 * (j + 1)],
                    out_offset=None,
                    in_=xtbl[:],
                    in_offset=IndirectOffsetOnAxis(ap=idst_sb[:, so + j : so + j + 1], axis=0),
                )
            gsT = sb.tile([128, 512], BF16, tag="gsT")
            gdT = sb.tile([128, 512], BF16, tag="gdT")
            for g, gT, eng in ((gs, gsT, "v"), (gd, gdT, "s")):
                tp = psT.tile([128, 512], BF16, tag="tp")
                for j in range(4):
                    nc.tensor.transpose(
                        tp[:, 128 * j : 128 * (j + 1)],
                        g[:, 128 * j : 128 * (j + 1)], T["ID_B"][:]
                    )
                if eng == "v":
                    nc.vector.tensor_copy(gT[:], tp[:])
                else:
                    nc.scalar.copy(gT[:], tp[:])

            # MLP1 (feat-major): P1 [h, 512]
            es = 512 * t
            zsrc = T["ZT0"] if l == 0 else T["ZT_D"]
            ztile = sb.tile([128, 512], BF16, tag="ztile")
            nc.sync.dma_start(ztile[:], zsrc[:, es : es + 512])
            if l < 3:
                ztnew = sb.tile([128, 512], BF16, tag="ztnew")
            p1 = ps.tile([128, 512], F32, tag="p1")
            nc.tensor.matmul(p1[:], wr(f"EB0{l}"), T["ONES_R"][:1, :], start=True, stop=False)
            nc.tensor.matmul(p1[:], wb(f"EW0S{l}"), gsT[:], start=False, stop=False)
            nc.tensor.matmul(p1[:], wb(f"EW0D{l}"), gdT[:], start=False, stop=False)
            nc.tensor.matmul(
                p1[:], wb(f"EW0Z{l}"), ztile[:], start=False, stop=True
            )

            t1 = self._elu(T, sb, p1, 512, "e")

            # MLP2 flipped -> natural [128e, 128h] blocks of one wide psum tile
            y_ps = psY.tile([128, 512], F32, tag="y")
            for j in range(4):
                yv = y_ps[:, 128 * j : 128 * (j + 1)]
                nc.tensor.matmul(
                    yv, T["ONES_R"][:1, :128], wr(f"EB1{l}"), start=True, stop=False
                )
                nc.tensor.matmul(
                    yv, t1[:, 128 * j : 128 * (j + 1)], wf(f"EW1{l}"),
                    start=False, stop=False,
                )
                nc.tensor.matmul(
                    yv, ztile[:, 128 * j : 128 * (j + 1)], wb(f"GDE{l}"),
                    start=False, stop=True,
                )

            # blocked LayerNorm over the 4 natural chunks at once
            sums = stat.tile([128, 4], F32, tag="sums")
            mus = stat.tile([128, 4], F32, tag="mus")
            rstds = stat.tile([128, 4], F32, tag="rstds")
            y_sb = sb.tile([128, 512], F32, tag="y_sb5")
            nc.scalar.copy(y_sb[:], y_ps[:])
            b3 = lambda ap: ap.rearrange("p (j f) -> p j f", j=4)
            c3 = lambda ap: ap.rearrange("p (j u) -> p j u", u=1).to_broadcast([128, 4, 128])
            nc.vector.tensor_reduce(
                sums[:].rearrange("p (j u) -> p j u", u=1), b3(y_sb[:]),
                axis=mybir.AxisListType.X, op=AX.add,
            )
            nc.scalar.mul(mus[:], sums[:], 1.0 / 128.0)
            yc = sb.tile([128, 512], F32, tag="yc5")
            nc.vector.tensor_tensor(b3(yc[:]), b3(y_sb[:]), c3(mus[:]), AX.subtract)
            sq = sb.tile([128, 512], F32, tag="sq5")
            nc.scalar.square(sq[:], yc[:])
            ssq = stat.tile([128, 4], F32, tag="ssq")
            nc.vector.tensor_reduce(
                ssq[:].rearrange("p (j u) -> p j u", u=1), b3(sq[:]),
                axis=mybir.AxisListType.X, op=AX.add,
            )
            std = stat.tile([128, 4], F32, tag="std")
            nc.scalar.activation(std[:], ssq[:], AF.Sqrt, bias=T["EPSB"][:, :], scale=1.0 / 128.0)
            nc.vector.reciprocal(rstds[:], std[:])
            z_sb = sb.tile([128, 512], BF16, tag="z_sb5")
            nc.vector.tensor_tensor(b3(z_sb[:]), b3(yc[:]), c3(rstds[:]), AX.mult)

            # blocked S one-hot for the 4 chunks
            S4 = sb.tile([128, 512], BF16, tag="S4")
            nc.vector.tensor_tensor(
                b3(S4[:]),
                T["IOTA_F"][:].rearrange("p (u f) -> p u f", u=1).to_broadcast([128, 4, 128]),
                c3(dloc_sb[:, so : so + 4]),
                AX.is_equal,
            )

            for j in range(4):
                ch = c0 + j  # global 128-chunk index
                in_seg = ch < c.e_chunks
                w = ch // c.T_w  # window
                s = ch % c.T_w
                z_ap = z_sb[:, 128 * j : 128 * (j + 1)]
                if in_seg:
                    if w not in agg_ps:
                        agg_ps[w] = psA.tile([128, 128], F32, tag="agg", name=f"agg_{l}_{w}")
                    nc.tensor.matmul(
                        agg_ps[w][:], S4[:, 128 * j : 128 * (j + 1)], z_ap,
                        start=(s == 0), stop=(s == c.T_w - 1),
                    )
                # window complete -> agg scale + transpose into AGGT
                if in_seg and s == c.T_w - 1:
                    agg_sb = sb.tile([128, 128], F32, tag="agg_sb")
                    nc.vector.tensor_scalar(
                        agg_sb[:], agg_ps[w][:],
                        T["RECIP_sb"][:, w : w + 1], None, AX.mult,
                    )
                    atp = psY.tile([128, 128], F32, tag="y")
                    nc.tensor.transpose(atp[:], agg_sb[:], T["ID_F"][:])
                    nc.scalar.copy(T["AGGT"][:, 128 * w : 128 * (w + 1)], atp[:])
                    del agg_ps[w]

            # zT writeback (skip on the very last edge layer)
            if l < 3:
                ztp = psT.tile([128, 512], BF16, tag="tp")
                for j in range(4):
                    nc.tensor.transpose(
                        ztp[:, 128 * j : 128 * (j + 1)],
                        z_sb[:, 128 * j : 128 * (j + 1)], T["ID_B"][:]
                    )
                nc.vector.tensor_copy(ztnew[:], ztp[:])
                nc.sync.dma_start(T["ZT_D"][:, es : es + 512], ztnew[:])

    # ------------------------------------------------------------------
    def _node_mlp(self, tc, T, l, sb, ps, psY, stat, n0, nw, zx_sb, use_agg,
                  z_nat_cb):
        """Shared node update for a tile of nw slots starting at n0.

        zx_sb: [128, nw] f32 feat-major carrier chunk in SBUF.
        z_nat_cb(chunk_j, nat_w, z_nat_sb_tile): consume LN output (natural f32).
        """
        nc = self.nc
        c = self.cfg
        wb, wf, wr = self._wb, self._wf, self._wr
        p1 = ps.tile([128, 512], F32, tag="p1")
        nc.tensor.matmul(p1[:, :nw], wr(f"NB0{l}"), T["ONES_R"][:1, :nw], start=True, stop=False)
        nc.tensor.matmul(p1[:, :nw], wf(f"NW0X{l}"), zx_sb[:, :nw], start=False, stop=not use_agg)
        if use_agg:
            nc.tensor.matmul(
                p1[:, :nw], wb(f"NW0A{l}"), T["AGGT"][:, n0 : n0 + nw],
                start=False, stop=False,
            )
            nc.tensor.matmul(
                p1[:, :nw],
                wr(f"NMASKV{l}"),
                T["BMASK_sb"][:1, n0 : n0 + nw],
                start=False, stop=True,
            )
        t1 = self._elu(T, sb, p1, nw, "n")
        n_ch = (nw + 127) // 128
        sums = stat.tile([128, 4], F32, tag="sums_n")
        mus = stat.tile([128, 4], F32, tag="mus_n")
        rstds = stat.tile([128, 4], F32, tag="rstds_n")
        for j in range(n_ch):
            cw = min(128, nw - 128 * j)
            y_ps = psY.tile([128, 128], F32, tag="y")
            nc.tensor.matmul(
                y_ps[:cw, :],
                T["ONES_R"][:1, :cw],
                wr(f"NB1{l}"), start=True, stop=False,
            )
            nc.tensor.matmul(
                y_ps[:cw, :], t1[:, 128 * j : 128 * j + cw], wf(f"NW1{l}"),
                start=False, stop=False,
            )
            nc.tensor.matmul(
                y_ps[:cw, :], zx_sb[:, 128 * j : 128 * j + cw], wf(f"GDX{l}"),
                start=False, stop=True,
            )
            z_nat = sb.tile([128, 128], F32, tag="zn_nat")
            self._ln_block(T, stat, sb, y_ps, cw, z_nat[:cw, :], sums, mus, rstds, j)
            z_nat_cb(j, cw, z_nat)

    def _carrier_writeback(self, T, sb, psT, zxt_new, j, cw, z_nat, ag_row0=None):
        """Transpose natural z chunk into feat-major carrier staging; optionally
        also emit bf16 natural rows into AGSRC (AllGather table source)."""
        nc = self.nc
        ztp = psT.tile([128, 128], F32, tag="tp")
        nc.tensor.transpose(ztp[:], z_nat[:, :], T["ID_F"][:])
        nc.vector.tensor_copy(zxt_new[:, 128 * j : 128 * j + cw], ztp[:, :cw])
        if ag_row0 is not None:
            agb = sb.tile([128, 128], BF16, tag="agb")
            nc.scalar.copy(agb[:cw, :], z_nat[:cw, :])
            nc.sync.dma_start(T["AGSRC"][ag_row0 + 128 * j : ag_row0 + 128 * j + cw, :], agb[:cw, :])

    def _node_phase_coarse(self, tc, T, l, sb, sb2, ps, psT, psY, stat):
        nc = self.nc
        c = self.cfg
        src_d = T["ZX0T"] if l == 0 else (T["ZXT_A"] if l == 1 else None)
        dst_d = T["ZXT_A"] if l == 0 else T["ZXT_B"]
        for n0 in range(0, c.n_slot, 512):
            nw = min(512, c.n_slot - n0)
            zx_sb = sb.tile([128, 512], F32, tag="zx_sb")
            nc.sync.dma_start(zx_sb[:, :nw], src_d[:, n0 : n0 + nw])
            zxt_new = sb.tile([128, 512], F32, tag="zxt_new")

            def cb(j, cw, z_nat, zxt_new=zxt_new, n0=n0):
                self._carrier_writeback(T, sb, psT, zxt_new, j, cw, z_nat, ag_row0=n0)

            self._node_mlp(tc, T, l, sb, ps, psY, stat, n0, nw, zx_sb, True, cb)
            nc.sync.dma_start(dst_d[:, n0 : n0 + nw], zxt_new[:, :nw])

    def _node_phase_fine(self, tc, T, l, sb, sb2, ps, psT, psY, stat, last):
        nc = self.nc
        c = self.cfg
        src_d = T["ZXFT_A"] if l == 2 else T["ZXFT_B"]
        dst_d = T["ZXFT_B"] if l == 2 else None
        # zero the AGGT tail once (inactive region up to next 512 boundary)
        if c.n_slot_pad > c.n_slot:
            nc.vector.memset(T["AGGT"][:, c.n_slot : c.n_slot_pad], 0.0)
        for n0 in range(0, c.f_slot, 512):
            nw = 512
            use_agg = n0 < c.n_slot
            zx_sb = sb.tile([128, 512], F32, tag="zx_sb")
            nc.sync.dma_start(zx_sb[:, :nw], src_d[:, n0 : n0 + nw])
            if last:
                zxt_new = sb.tile([128, 512], F32, tag="zxt_new")

                def cb(j, cw, z_nat, zxt_new=zxt_new):
                    self._carrier_writeback(T, sb, psT, zxt_new, j, cw, z_nat)

                self._node_mlp(tc, T, l, sb, ps, psY, stat, n0, nw, zx_sb, use_agg, cb)
                # output head: out = elu(zxT' @ OWP + OBP) - 1
                for j in range(4):
                    op = psY.tile([128, OUT_DIM], F32, tag="y")
                    nc.tensor.matmul(
                        op[:],
                        T["ONES_R"][:1, :128],
                        T["OBP_sb"][:1, :], start=True, stop=False,
                    )
                    nc.tensor.matmul(
                        op[:], zxt_new[:, 128 * j : 128 * (j + 1)], T["OWP_sb"][:],
                        start=False, stop=True,
                    )
                    m = sb.tile([128, OUT_DIM], F32, tag="om")
                    nc.vector.tensor_scalar(m[:], op[:], 1.0, None, AX.min)
                    e = sb.tile([128, OUT_DIM], F32, tag="oe")
                    nc.scalar.activation(e[:], m[:], AF.Exp, bias=T["NEG1"][:, :], scale=1.0)
                    o = sb.tile([128, OUT_DIM], F32, tag="oo")
                    nc.vector.tensor_tensor(o[:], e[:], op[:], AX.max)
                    of = sb.tile([128, OUT_DIM], F32, tag="of")
                    nc.vector.tensor_scalar(of[:], o[:], 1.0, None, AX.subtract)
                    nc.sync.dma_start(
                        T["OUT"][n0 + 128 * j : n0 + 128 * (j + 1), :], of[:]
                    )
            else:
                zxt_new = sb.tile([128, 512], F32, tag="zxt_new")

                def cb(j, cw, z_nat, zxt_new=zxt_new, n0=n0):
                    act = (n0 + 128 * j) < c.n_slot
                    self._carrier_writeback(
                        T, sb, psT, zxt_new, j, cw, z_nat,
                        ag_row0=n0 if act else None,
                    )

                self._node_mlp(tc, T, l, sb, ps, psY, stat, n0, nw, zx_sb, use_agg, cb)
                nc.sync.dma_start(dst_d[:, n0 : n0 + nw], zxt_new[:, :nw])

    # ------------------------------------------------------------------
    def _upsample(self, tc, T, sb, sb2, ps, psT, psY, stat):
        nc = self.nc
        c = self.cfg
        for t in range(c.f_slot // 512):
            n0 = 512 * t
            c0 = n0 // 128
            pct = sb.tile([2, 512], F32, tag="pct")
            pft = sb.tile([2, 512], F32, tag="pft")
            nc.sync.dma_start(pct[:], T["PCT"][:, n0 : n0 + 512])
            nc.sync.dma_start(pft[:], T["PFT"][:, n0 : n0 + 512])
            dT = sb.tile([2, 512], F32, tag="dT")
            nc.vector.tensor_tensor(dT[:], pct[:], pft[:], AX.subtract)
            # d-MLP
            p = ps.tile([128, 512], F32, tag="p1")
            nc.tensor.matmul(p[:], self._wr("DB0"), T["ONES_R"][:1, :], start=True, stop=False)
            nc.tensor.matmul(p[:], T["DW0_sb"][:], dT[:], start=False, stop=True)
            e1 = self._elu(T, sb, p, 512, "u1")
            p2 = ps.tile([128, 512], F32, tag="p1")
            nc.tensor.matmul(p2[:], self._wr("DB1"), T["ONES_R"][:1, :], start=True, stop=False)
            nc.tensor.matmul(p2[:], self._wf("DW1"), e1[:], start=False, stop=True)
            eacT = sb.tile([128, 512], F32, tag="eacT")
            m = sb.tile([128, 512], F32, tag="elu_m")
            nc.vector.tensor_scalar(m[:], p2[:], 1.0, None, AX.min)
            ex = sb.tile([128, 512], F32, tag="elu_e")
            nc.scalar.activation(ex[:], m[:], AF.Exp, bias=T["NEG1"][:, :], scale=1.0)
            nc.vector.tensor_tensor(eacT[:], ex[:], p2[:], AX.max)

            # xg gather
            icl = sb2.tile([128, 4], I32, tag="icl")
            nc.sync.dma_start(icl[:], T["IDX_CL"][:, c0 : c0 + 4])
            g = sb.tile([128, 4 * 128], BF16, tag="g_cl")
            for j in range(4):
                if os.environ.get("KNOGATHER"):
                    nc.sync.dma_start(g[:, 128 * j : 128 * (j + 1)], T["TBL"][0:128, :])
                    continue
                nc.gpsimd.indirect_dma_start(
                    out=g[:, 128 * j : 128 * (j + 1)],
                    out_offset=None,
                    in_=T["TBL"][:],
                    in_offset=IndirectOffsetOnAxis(ap=icl[:, j : j + 1], axis=0),
                )
            xgT = sb.tile([128, 512], BF16, tag="xgT")
            for j in range(4):
                tp = psT.tile([128, 128], BF16, tag="tp")
                nc.tensor.transpose(tp[:], g[:, 128 * j : 128 * (j + 1)], T["ID_B"][:])
                if j % 2 == 0:
                    nc.vector.tensor_copy(xgT[:, 128 * j : 128 * (j + 1)], tp[:])
                else:
                    nc.scalar.copy(xgT[:, 128 * j : 128 * (j + 1)], tp[:])

            # u-MLP1
            pu = ps.tile([128, 512], F32, tag="p1")
            nc.tensor.matmul(pu[:], self._wr("UB0"), T["ONES_R"][:1, :], start=True, stop=False)
            nc.tensor.matmul(pu[:], self._wf("UW0E"), eacT[:], start=False, stop=False)
            nc.tensor.matmul(pu[:], self._wb("UW0X"), xgT[:], start=False, stop=True)
            t1u = self._elu(T, sb, pu, 512, "u3")

            zxt_new = sb.tile([128, 512], F32, tag="zxt_new")
            sums = stat.tile([128, 4], F32, tag="sums_u")
            mus = stat.tile([128, 4], F32, tag="mus_u")
            rstds = stat.tile([128, 4], F32, tag="rstds_u")
            for j in range(4):
                yu = psY.tile([128, 128], F32, tag="y")
                nc.tensor.matmul(
                    yu[:],
                    T["ONES_R"][:1, :128],
                    self._wr("UB1"), start=True, stop=False,
                )
                nc.tensor.matmul(
                    yu[:], t1u[:, 128 * j : 128 * (j + 1)], self._wf("UW1"),
                    start=False, stop=True,
                )
                # t2u' natural via elu on psum
                mj = sb.tile([128, 128], F32, tag="mj")
                nc.vector.tensor_scalar(mj[:], yu[:], 1.0, None, AX.min)
                ej = sb.tile([128, 128], F32, tag="ej")
                nc.scalar.activation(ej[:], mj[:], AF.Exp, bias=T["NEG1"][:, :], scale=1.0)
                t2u = sb.tile([128, 128], F32, tag="t2u")
                nc.vector.tensor_tensor(t2u[:], ej[:], yu[:], AX.max)
                # residual: y = t2u' + eac_nat'  (constant shifts cancel in LN)
                etp = psT.tile([128, 128], F32, tag="tp")
                nc.tensor.transpose(etp[:], eacT[:, 128 * j : 128 * (j + 1)], T["ID_F"][:])
                y_sb = sb.tile([128, 128], F32, tag="y_sb")
                nc.vector.tensor_tensor(y_sb[:], t2u[:], etp[:], AX.add)
                nc.vector.tensor_reduce(sums[:, j : j + 1], y_sb[:], axis=mybir.AxisListType.X, op=AX.add)
                nc.scalar.mul(mus[:, j : j + 1], sums[:, j : j + 1], 1.0 / 128.0)
                sq = sb.tile([128, 128], F32, tag="sq_scratch")
                ssq = stat.tile([128, 1], F32, tag="ssq")
                nc.scalar.activation(
                    sq[:], y_sb[:], AF.Square,
                    bias=mus[:, j : j + 1], scale=-1.0,
                )
                nc.vector.tensor_reduce(ssq[:], sq[:], axis=mybir.AxisListType.X, op=AX.add)
                std = stat.tile([128, 1], F32, tag="std")
                nc.scalar.activation(std[:], ssq[:], AF.Sqrt, bias=T["EPSB"][:, :], scale=1.0 / 128.0)
                nc.vector.reciprocal(rstds[:, j : j + 1], std[:])
                z_nat = sb.tile([128, 128], F32, tag="zn_nat")
                nc.vector.tensor_scalar(
                    z_nat[:], y_sb[:], mus[:, j : j + 1], rstds[:, j : j + 1],
                    AX.subtract, AX.mult,
                )
                act = (n0 + 128 * j) < c.n_slot
                self._carrier_writeback(
                    T, sb, psT, zxt_new, j, 128, z_nat,
                    ag_row0=n0 if act else None,
                )
            nc.sync.dma_start(T["ZXFT_A"][:, n0 : n0 + 512], zxt_new[:])

    # ------------------------------------------------------------------
    def _allgather(self, tc, T):
        nc = self.nc
        c = self.cfg
        if os.environ.get("KNOAG"):
            nc.sync.dma_start(T["TBL"][0 : c.n_slot, :], T["AGSRC"][:])
            return
        nc.gpsimd.collective_compute(
            "AllGather",
            AX.bypass,
            replica_groups=[list(range(c.n_cores))],
            ins=[T["AGSRC"][:]],
            outs=[T["TBL"][:]],
        )


# ----------------------------------------------------------------------------
# Entry point
# ----------------------------------------------------------------------------

_PROG_CACHE = {}


def _get_prog(cfg: Cfg):
    key = (cfg.n_cores, cfg.W, cfg.T_w, cfg.n_inact_slot)
    if key not in _PROG_CACHE:
        _PROG_CACHE[key] = Prog(cfg)
    return _PROG_CACHE[key]


def kernel(**inputs):
    cfg = FULL
    n_c = inputs["x"].shape[0]
    e_c = inputs["edge_index_c"].shape[1]
    n_f = inputs["clusters"].shape[0]
    folds = _fold_weights(inputs, cfg)
    part = _partition(inputs, cfg, n_c, n_f, e_c)
    core_inputs, out_map = _make_core_inputs(inputs, cfg, part, folds)
    prog = _get_prog(cfg)
    res = run_bass_kernel_spmd(
        prog.nc, core_inputs, list(range(cfg.n_cores))
    )
    out = np.zeros((n_f, OUT_DIM), np.float32)
    for c in range(cfg.n_cores):
        o = res.results[c]["OUT"]
        m = out_map[c] >= 0
        out[out_map[c][m]] = o[m]
    return (out, inputs["edge_index_c"])
